# revision 12
# baseline (speedup 1.0000x reference)
"""CRR binomial-tree American put pricer on Trainium2 (Bass/Tile).

Math (per batch element, faithful to the reference):
  dt = T/n; u = exp(sigma*sqrt(dt)); d = 1/u
  p = clip((exp(R*dt) - d) / (u - d + 1e-8), 0, 1); disc = exp(-R*dt)
  terminal V[j] = max(K - S*u^j*d^(n-j), 0),  j = 0..n
  backward:  V[j] = max(disc*(p*V[j+1] + (1-p)*V[j]), K - S_s[j])

Device formulation (batch on partitions, per-partition scalars):
  a = disc*p, b = disc*(1-p), r = a/b = p/(1-p)
  On the CRR lattice S_s[j] = u^(n-s) * S_n[j], so the terminal spot grid
  S_n stays read-only in SBUF and a per-element running power
  muPow = -u^(n-s) (updated once per step for ALL groups with one tiny
  [128, G] tensor_tensor) provides the exercise values with full relative
  precision (mixing K into a recurrence would destroy tiny spots via
  cancellation).
  step:  muPow *= u                    (DVE tt on [128, G], negligible)
         E = S_n*muPow + K             (ACT Identity: scale/bias APs)
         T = (V_up * r) + V            (DVE scalar_tensor_tensor)
         V = (T * b) max E             (DVE scalar_tensor_tensor)
  ACT work is hidden behind the two wide DVE ops.

Sharding: pure data parallel, batch 32768 -> 8 cores x 4096.
Each core: 4096 = 32 groups x 128 partitions; V/S state [128, 32, 513]
stays resident in SBUF for the whole 512-step recursion.
"""

import numpy as np

N_STEPS = 512
RATE = 0.03
N_CORES = 8
P = 128

_cache = {}


def _host_constants(S, K, sigma, T):
    """Per-element scalar constants, computed in float64 then cast."""
    S = S.astype(np.float64)
    K64 = K.astype(np.float64)
    sigma = sigma.astype(np.float64)
    T = T.astype(np.float64)
    n = N_STEPS
    dt = T / n
    x = sigma * np.sqrt(dt)          # sigma*sqrt(dt) = ln(u)
    u = np.exp(x)
    d = 1.0 / u
    exp_rdt = np.exp(RATE * dt)
    p = np.clip((exp_rdt - d) / (u - d + 1e-8), 0.0, 1.0)
    disc = np.exp(-RATE * dt)
    b = disc * (1.0 - p)
    r = p / (1.0 - p)
    lnSd = np.log(S) - n * x         # ln(S * d^n)
    f32 = lambda a: np.ascontiguousarray(a, dtype=np.float32)
    # only 5 arrays shipped to the device; u = exp(x), w = 2x and mu = -u
    # are derived on-device (cuts host->device transfer by 2/7)
    consts = dict(r=f32(r), b=f32(b), K=f32(K64), x=f32(x), lnSd=f32(lnSd))
    moneyness = np.log(K64 / S) / x  # c: node j at level s is ITM iff 2j-s < c
    return consts, moneyness


MARGIN = 24
# Steps with width <= TAIL_W are fused across all groups into 6 wide
# tensor_tensor ops (broadcast per-group constants). This amortizes the
# per-instruction overhead of tiny ops AND keeps the DVE instruction count
# below 32768 -- semaphore wait values appear to wrap at 15 bits, which
# silently corrupts synchronization (verified empirically: N=496*G=32*2
# = 31.7k DVE instrs works, 512*32*2 = 32.8k fails).
TAIL_W = 96


KAPPA = 4.0
PMIN = 0.48   # provable lower bound on p for the input parameter ranges


def _band_schedules(c_sorted, n_steps, groups, lanes_per_group):
    """Per-global-group L (skip-prefix: V==exercise below, patched),
    Wex (max-window top: exercise < 0 above) and U (zero-region cap)
    schedules.

    The cap: a node (s, j) is worth K*P[ITM terminal reachable] <=
    K*P[Binom(m, p) < jex_hi - j], m = n-s.  For j >= jex_hi - p*m +
    KAPPA*sqrt(m) that tail is < ~1e-13 K, so those nodes are dead
    weight and the recurrence skips them.  U is monotone (U(s) <=
    U(s+1)); the strip [U(s), U(s+1)) is zeroed when U drops so the
    boundary read V[U] sees ~0 instead of a stale frozen value."""
    n = n_steps
    L = np.zeros((groups, n), dtype=np.int32)
    We = np.zeros((groups, n), dtype=np.int32)
    U = np.zeros((groups, n), dtype=np.int32)
    for g in range(groups):
        cmin = c_sorted[g * lanes_per_group]
        cmax = c_sorted[(g + 1) * lanes_per_group - 1]
        jex_n = min(int(np.floor((n + cmin) / 2.0)), n)
        jex_hi = int(np.ceil((n + cmax) / 2.0))
        uprev = n + 1
        for s in range(n - 1, -1, -1):
            W = s + 1
            m = n - s
            lraw = jex_n - m - MARGIN
            L[g, s] = max(0, 16 * (lraw // 16))
            wraw = int(np.ceil((s + cmax) / 2.0)) + MARGIN
            We[g, s] = min(W, max(0, wraw))
            if We[g, s] < L[g, s]:
                We[g, s] = L[g, s]
            cut = max(0, int(np.floor(PMIN * m - KAPPA * np.sqrt(m))))
            # never clip inside the fused tail's [0, W) rectangle (the
            # tail ops span all groups at full width)
            uvalid = max(jex_hi + 1 - cut, TAIL_W + 2)
            u_ = 16 * int(np.ceil(uvalid / 16.0))
            u_ = min(W, max(u_, We[g, s] + 1), uprev)
            U[g, s] = max(u_, L[g, s] + 1)
            uprev = U[g, s]
    return L, We, U


def _build(n_steps, groups, bands=None):
    """Build the Bass/Tile program for one core (n_per_core = groups*128).

    bands: optional (L, We) int arrays [groups, n_steps]. For step s and
    group g, the combine runs on [L, W), the exercise max on [L, We) and a
    plain b-scale on [We, W); V[j] for j < L equals the exercise value and
    is patched in 16-wide chunks as L steps down.
    """
    import concourse.bacc as bacc
    import concourse.tile as tile
    from concourse import mybir

    W0 = n_steps + 1
    BC = groups * P
    f32 = mybir.dt.float32
    Alu = mybir.AluOpType
    Act = mybir.ActivationFunctionType

    nc = bacc.Bacc("TRN2", target_bir_lowering=False, debug=False)

    tail_w = min(TAIL_W, n_steps)
    # Choose the minimum width-saving threshold for emitting the separate
    # b-scale op (3 DVE ops instead of 2) that keeps the DVE instruction
    # count below the 15-bit semaphore-safe budget.
    split_thresh = None
    if bands is not None:
        # + ~600: cap-strip/terminal memsets and derived-constant setup
        base = (n_steps - tail_w) * groups * 2 + tail_w * 6 + n_steps + 800
        for cand in (116, 160, 240, 360, 10 ** 9):
            n_split = sum(
                1
                for s in range(tail_w, n_steps)
                for g in range(groups)
                if bands[1][g, s] > bands[0][g, s]
                and min(int(bands[2][g, s]), s + 1) - bands[1][g, s] > cand
            )
            if base + n_split <= 31000:
                split_thresh = cand
                break

    ins = {name: nc.dram_tensor(name, [BC], f32, kind="ExternalInput")
           for name in ("r", "b", "K", "x", "lnSd")}
    out_d = nc.dram_tensor("out", [BC], f32, kind="ExternalOutput")

    with tile.TileContext(nc) as tc:
        with (
            tc.tile_pool(name="state", bufs=1) as state,
            tc.tile_pool(name="tmp", bufs=4) as tmp,
        ):
            V = state.tile([P, groups, W0], f32)
            Sp = state.tile([P, groups, W0], f32)
            jb = state.tile([P, W0], f32)
            jbi = state.tile([P, W0], mybir.dt.int32)
            price = state.tile([P, groups], f32)
            mupow = state.tile([P, groups], f32)
            mupowp = state.tile([P, groups], f32)
            scal = {name: state.tile([P, groups], f32, name=f"scal_{name}")
                    for name in ins}
            for name in ("w", "u", "mu"):
                scal[name] = state.tile([P, groups], f32, name=f"scal_{name}")

            nc.gpsimd.iota(jbi, pattern=[[1, W0]], base=0,
                           channel_multiplier=0)
            nc.vector.tensor_copy(jb, jbi)
            for name, dram in ins.items():
                nc.sync.dma_start(
                    out=scal[name],
                    in_=dram[:].rearrange("(g p) -> p g", p=P),
                )
            # derived on-device: w = 2x, u = exp(x), mu = -u
            nc.vector.tensor_scalar(
                out=scal["w"], in0=scal["x"], scalar1=2.0, scalar2=None,
                op0=Alu.mult)
            nc.scalar.activation(scal["u"], scal["x"], Act.Exp)
            nc.vector.tensor_scalar(
                out=scal["mu"], in0=scal["u"], scalar1=-1.0, scalar2=None,
                op0=Alu.mult)

            sc = {name: (lambda t: (lambda g: t[:, g:g + 1]))(t)
                  for name, t in scal.items()}

            # terminal: S_T into the spot state; V = max(K - S_T, 0)
            nc.vector.tensor_copy(mupow, scal["mu"])
            nc.vector.memset(mupowp, -1.0)
            for g in range(groups):
                nc.scalar.activation(Sp[:, g, :], jb, Act.Exp,
                                     bias=sc["lnSd"](g), scale=sc["w"](g))
                e0 = tmp.tile([P, W0], f32, tag="E")
                nc.scalar.activation(e0, Sp[:, g, :], Act.Identity,
                                     bias=sc["K"](g), scale=-1.0)
                nc.vector.tensor_scalar(
                    out=V[:, g, :], in0=e0, scalar1=0.0,
                    scalar2=None, op0=Alu.max)
                if bands is not None:
                    u0 = int(bands[2][g, n_steps - 1])
                    if u0 <= n_steps:
                        # cap boundary read sees ~0, not the payoff
                        nc.vector.memset(V[:, g, u0:u0 + 1], 0.0)

            # backward induction; muPow = -u^(n-s) at step s, muPowp one
            # step behind (= -u^(n-s-1), the level-(s+1) exercise scale).
            for s in range(n_steps - 1, tail_w - 1, -1):
                W = s + 1
                for g in range(groups):
                    if bands is None:
                        L, We, Ue = 0, W, W
                    else:
                        L = int(bands[0][g, s])
                        We = int(bands[1][g, s])
                        Ue = min(int(bands[2][g, s]), W)
                        if We > L and Ue - We <= split_thresh:
                            We = Ue  # single stt2 over [L, Ue)
                        if s < n_steps - 1:
                            Un, Up = int(bands[2][g, s]), int(bands[2][g, s + 1])
                            if Un < Up:
                                # zero the dropped cap strip before reading
                                nc.vector.memset(V[:, g, Un:Up], 0.0)
                            Lp = int(bands[0][g, s + 1])
                            if L < Lp:
                                # V[L:Lp] is stale; it equals the level-(s+1)
                                # exercise value K + muPowp * S_n.
                                nc.scalar.activation(
                                    V[:, g, L:Lp], Sp[:, g, L:Lp],
                                    Act.Identity, bias=sc["K"](g),
                                    scale=mupowp[:, g:g + 1])
                    if We > L:
                        e_t = tmp.tile([P, We - L], f32, tag="E")
                        nc.scalar.activation(e_t, Sp[:, g, L:We], Act.Identity,
                                             bias=sc["K"](g),
                                             scale=mupow[:, g:g + 1])
                    t_t = tmp.tile([P, Ue - L], f32, tag="T")
                    nc.vector.scalar_tensor_tensor(
                        out=t_t, in0=V[:, g, L + 1:Ue + 1], scalar=sc["r"](g),
                        in1=V[:, g, L:Ue], op0=Alu.mult, op1=Alu.add)
                    if We > L:
                        out_ap = (price[:, g:g + 1] if s == 0
                                  else V[:, g, L:We])
                        nc.vector.scalar_tensor_tensor(
                            out=out_ap, in0=t_t[:, 0:We - L],
                            scalar=sc["b"](g),
                            in1=e_t, op0=Alu.mult, op1=Alu.max)
                    if We < Ue:
                        out_ap = (price[:, g:g + 1] if s == 0
                                  else V[:, g, We:Ue])
                        nc.vector.tensor_scalar(
                            out=out_ap, in0=t_t[:, We - L:Ue - L],
                            scalar1=sc["b"](g), scalar2=None, op0=Alu.mult)
                if s > 0:
                    if bands is not None:
                        nc.vector.tensor_copy(mupowp, mupow)
                    nc.vector.tensor_tensor(
                        out=mupow, in0=mupow, in1=scal["u"],
                        op=Alu.mult)

            # entering the fused tail: patch any still-stale prefix of V
            # (below the band's L at the last banded step) with the
            # level-(tail_w) exercise values K + muPowp * S_n.
            if bands is not None and tail_w < n_steps:
                for g in range(groups):
                    Lt = int(bands[0][g, tail_w])
                    if Lt > 0:
                        nc.scalar.activation(
                            V[:, g, 0:Lt], Sp[:, g, 0:Lt], Act.Identity,
                            bias=sc["K"](g), scale=mupowp[:, g:g + 1])

            # fused tail: all groups in one 6-op step (full width, tt ops
            # with broadcast [P, groups, 1] constants).
            rb3 = scal["r"][:, :, None]
            bb3 = scal["b"][:, :, None]
            kb3 = scal["K"][:, :, None]
            mp3 = mupow[:, :, None]
            for s in range(tail_w - 1, -1, -1):
                W = s + 1
                sh = [P, groups, W]
                e3 = tmp.tile([P, groups, tail_w], f32, tag="E3", bufs=1)
                t3 = tmp.tile([P, groups, tail_w], f32, tag="T3", bufs=1)
                e3 = e3[:, :, 0:W]
                t3 = t3[:, :, 0:W]
                nc.vector.tensor_tensor(
                    out=e3, in0=Sp[:, :, 0:W], in1=mp3.to_broadcast(sh),
                    op=Alu.mult)
                nc.vector.tensor_tensor(
                    out=e3, in0=e3, in1=kb3.to_broadcast(sh), op=Alu.add)
                nc.vector.tensor_tensor(
                    out=t3, in0=V[:, :, 1:W + 1], in1=rb3.to_broadcast(sh),
                    op=Alu.mult)
                nc.vector.tensor_tensor(
                    out=t3, in0=t3, in1=V[:, :, 0:W], op=Alu.add)
                nc.vector.tensor_tensor(
                    out=t3, in0=t3, in1=bb3.to_broadcast(sh), op=Alu.mult)
                out3 = price[:, :, None] if s == 0 else V[:, :, 0:W]
                nc.vector.tensor_tensor(out=out3, in0=t3, in1=e3, op=Alu.max)
                if s > 0:
                    nc.vector.tensor_tensor(
                        out=mupow, in0=mupow, in1=scal["u"], op=Alu.mult)

            nc.sync.dma_start(
                out=out_d[:].rearrange("(g p) -> p g", p=P), in_=price)

    nc.compile()
    return nc


def _in_maps(consts, n_steps, n_per_core, perm=None):
    maps = []
    for c in range(N_CORES):
        if perm is None:
            sel = slice(c * n_per_core, (c + 1) * n_per_core)
        else:
            sel = perm[c]
        m = {name: np.ascontiguousarray(arr[sel])
             for name, arr in consts.items()}
        maps.append(m)
    return maps


def _core_perm(order, batch):
    """Interleave the sorted order so that every core's group g covers the
    same moneyness quantile (sorted position q -> global group q//(8*128),
    core (q%1024)//128, lane q%128); returns [n_cores, n_per_core] index."""
    q = np.arange(batch)
    ggroup = q // (N_CORES * P)
    core = (q % (N_CORES * P)) // P
    lane = q % P
    slot = ggroup * P + lane
    perm = np.empty((N_CORES, batch // N_CORES), dtype=np.int64)
    perm[core, slot] = order
    return perm


def kernel(S, K, sigma, T):
    import os

    batch = S.shape[0]
    assert batch % (N_CORES * P) == 0, batch
    n_per_core = batch // N_CORES
    groups = n_per_core // P

    consts, c = _host_constants(S, K, sigma, T)
    banded = os.environ.get("BASS_BINOMIAL_NO_BAND") != "1"

    if banded:
        order = np.argsort(c)
        perm = _core_perm(order, batch)
        c_sorted = c[order]
        bands = _band_schedules(c_sorted, N_STEPS, groups, N_CORES * P)
        key = (N_STEPS, groups, bands[0].tobytes(), bands[1].tobytes(),
               bands[2].tobytes())
    else:
        perm, bands = None, None
        key = (N_STEPS, groups)

    if key not in _cache:
        _cache[key] = _build(N_STEPS, groups, bands)
    nc = _cache[key]

    from concourse.bass_utils import run_bass_kernel_spmd

    maps = _in_maps(consts, N_STEPS, n_per_core, perm)
    res = run_bass_kernel_spmd(nc, maps, list(range(N_CORES))).results
    out = np.empty(batch, dtype=np.float32)
    for core in range(N_CORES):
        if perm is None:
            out[core * n_per_core:(core + 1) * n_per_core] = res[core]["out"]
        else:
            out[perm[core]] = res[core]["out"]
    return out

